# revision 40
# baseline (speedup 1.0000x reference)
"""Multi-head attention (B=8, T=2048, D=512, H=8) on 8 TRN2 NeuronCores.

Sharding: data-parallel over batch — one batch element per core, no
collectives. Host-side prep (part of shard/unshard): transpose x inputs to
[D, T], cast matmul operands to bf16, pass (1 - mask)^T chunk-major, and
transpose the per-core output y^T back to [T, D].

Per-core algorithm ("transposed flash"), v2 — head-pair-packed scores:
  P1: Q^T = Wq x^T and K^T = Wk x^T stored pair-major ([128, T] tiles, head
      2j in rows 0-63, head 2j+1 in rows 64-127 — no zero padding);
      V = x Wv^T augmented with a ones column per head (softmax denom).
  P2: per (q-block of 512, head-pair j, t2-chunk c):
        two 64-contraction score matmuls run CONCURRENTLY on the PE via
        tile_position (0,0)/(64,0), writing the two halves of one
        [128, 1024] PSUM tile; one exp ACTIVATE covers the pair; one DVE
        multiply applies the (1-mask) chunk (duplicated into both halves);
        per-head attnV accumulation into [65, 512] PSUM (row 64 = denom).
      epilogue per head: recip(denom) via 8-partition split + DRAM bounce
      broadcast, multiply into o2 SBUF.
  P3: y^T = Wo^T.T @ O^T (+bo) per q-block, DMA out.

PSUM budget (8 banks): tag s = 2x[128,1024] (4), tag o = 2x[65,512] (2),
tag w = 2x[128,512] (2, dedicated to projections + P3 so they never starve
the softmax pipeline). K/Q projections for j>0 are interleaved into P2 in
4-matmul quanta; P3(qb) is interleaved into block (qb+1, 0).
"""

import numpy as np
import ml_dtypes

B, T, FDIM, H = 8, 2048, 512, 8
DK = FDIM // H          # 64
NFT = FDIM // 128       # 4 fo-tiles
NCH = T // 128          # 16 t2-chunks
QB = 4                  # q blocks
QBS = T // QB           # 512
N_CORES = 8

BF16 = ml_dtypes.bfloat16

_cache = {}


def _build_nc():
    import concourse.bass as bass
    import concourse.mybir as mybir
    from concourse import bacc, tile

    f32 = mybir.dt.float32
    bf16 = mybir.dt.bfloat16
    Exp = mybir.ActivationFunctionType.Exp
    Alu = mybir.AluOpType

    nc = bacc.Bacc("TRN2", target_bir_lowering=False, debug=False,
                   num_devices=N_CORES)

    # DRAM I/O (per-core shard shapes)
    xqT = nc.dram_tensor("xqT", [FDIM, T], bf16, kind="ExternalInput")
    xkT = nc.dram_tensor("xkT", [FDIM, T], bf16, kind="ExternalInput")
    xvT = nc.dram_tensor("xvT", [FDIM, T], bf16, kind="ExternalInput")
    wqT = nc.dram_tensor("wqT", [FDIM, FDIM], bf16, kind="ExternalInput")
    wkT = nc.dram_tensor("wkT", [FDIM, FDIM], bf16, kind="ExternalInput")
    wvT = nc.dram_tensor("wvT", [FDIM, FDIM], bf16, kind="ExternalInput")
    woT = nc.dram_tensor("woT", [FDIM, FDIM], bf16, kind="ExternalInput")
    bq = nc.dram_tensor("bq", [FDIM], f32, kind="ExternalInput")
    bk = nc.dram_tensor("bk", [FDIM], f32, kind="ExternalInput")
    bv = nc.dram_tensor("bv", [FDIM], f32, kind="ExternalInput")
    bo = nc.dram_tensor("bo", [FDIM], f32, kind="ExternalInput")
    mbar = nc.dram_tensor("mbar", [NCH, 128, T], bf16, kind="ExternalInput")
    yT = nc.dram_tensor("yT", [FDIM, T], f32, kind="ExternalOutput")

    with tile.TileContext(nc) as tc:
        with (
            tc.tile_pool(name="consts", bufs=1) as consts,
            tc.tile_pool(name="qt", bufs=1) as qt_pool,
            tc.tile_pool(name="kt", bufs=1) as kt_pool,
            tc.tile_pool(name="vaug", bufs=1) as vaug_pool,
            tc.tile_pool(name="osb", bufs=1) as osb_pool,
            tc.tile_pool(name="ysb", bufs=1) as ysb_pool,
        ):
            # ---- consts: weights + biases (wv/xv first: head of critical path)
            wv_sb = [consts.tile([128, FDIM], bf16, tag=f"wv{fc}", name=f"wv{fc}") for fc in range(4)]
            wk_sb = [consts.tile([128, FDIM], bf16, tag=f"wk{fc}", name=f"wk{fc}") for fc in range(4)]
            wq_sb = [consts.tile([128, FDIM], bf16, tag=f"wq{fc}", name=f"wq{fc}") for fc in range(4)]
            wo_sb = [consts.tile([128, FDIM], bf16, tag=f"wo{j}", name=f"wo{j}") for j in range(NFT)]

            ones_sb = consts.tile([128, DK], bf16, tag="ones", name="ones")
            bq_sb = consts.tile([128, NFT], f32, tag="bq", name="bq")
            bk_sb = consts.tile([128, NFT], f32, tag="bk", name="bk")
            bo_sb = consts.tile([128, NFT], f32, tag="bo", name="bo")
            bv_bcast = consts.tile([128, FDIM], f32, tag="bv_bcast", name="bv_bcast")

            # ---- persistent activation tiles ----
            qT_sb = [qt_pool.tile([128, T], bf16, tag=f"qT{j}", name=f"qT{j}") for j in range(NFT)]
            kT_sb = [kt_pool.tile([128, T], bf16, tag=f"kT{j}", name=f"kT{j}") for j in range(NFT)]
            vaug = [vaug_pool.tile([128, H * (DK + 1)], bf16, tag=f"va{tt}", name=f"va{tt}")
                    for tt in range(NCH)]
            o2_sb = {}
            for qb in range(QB):
                for j in range(NFT):
                    o2_sb[(qb, j)] = osb_pool.tile([128, QBS], bf16, tag=f"o2_{qb}_{j}",
                                                   name=f"o2_{qb}_{j}")

            with (
                tc.tile_pool(name="xt", bufs=12) as xt_pool,
                tc.tile_pool(name="mask", bufs=20) as mask_pool,
                tc.tile_pool(name="praw", bufs=4) as praw_pool,
                tc.tile_pool(name="pm", bufs=5) as pm_pool,
                tc.tile_pool(name="rb", bufs=1) as rb_pool,
                tc.tile_pool(name="psum", bufs=2, space="PSUM") as psum_pool,
            ):
                # ============ DMA: inputs in critical-path order ============
                def load_x(x_dram, nm):
                    tiles = []
                    for fc in range(4):
                        xt = xt_pool.tile([128, T], bf16, tag="x", bufs=12, name=nm)
                        nc.sync.dma_start(out=xt[:], in_=x_dram[fc * 128:(fc + 1) * 128, :])
                        tiles.append(xt)
                    return tiles

                # Sync queue: K then V inputs.  Scalar queue (the second
                # HWDGE engine, idle during the head) issues Q inputs,
                # biases, Wo and the first q-block's masks in parallel —
                # DMA issue costs ~0.65us apiece, so one queue would
                # serialize ~35us of issue time.
                for fc in range(4):
                    nc.sync.dma_start(out=wv_sb[fc][:], in_=wvT[fc * 128:(fc + 1) * 128, :])
                xts_v = load_x(xvT, "xv")
                nc.sync.dma_start(
                    out=bv_bcast[:],
                    in_=bv.ap().rearrange("(a f) -> a f", a=1).to_broadcast([128, FDIM]))
                for fc in range(4):
                    nc.sync.dma_start(out=wk_sb[fc][:], in_=wkT[fc * 128:(fc + 1) * 128, :])
                xts_k = load_x(xkT, "xk")
                for fc in range(4):
                    nc.sync.dma_start(out=wq_sb[fc][:], in_=wqT[fc * 128:(fc + 1) * 128, :])
                xts_q = load_x(xqT, "xq")

                # ones column per head in V_aug
                nc.vector.memset(ones_sb[:], 1.0)
                for tt in range(NCH):
                    va = vaug[tt][:].rearrange("p (h d) -> p h d", d=DK + 1)
                    nc.vector.memset(va[:, :, DK:DK + 1], 1.0)

                # ============ compute helpers ============
                def v_proj_pair(t0):
                    # two V-proj tiles through one [128,1024] "s" slot
                    ps = psum_pool.tile([128, 1024], mybir.dt.float32,
                                        tag="s", name="vp")
                    for k, tt in enumerate((t0, t0 + 1)):
                        for fc in range(4):
                            nc.tensor.matmul(
                                ps[:, k * 512:(k + 1) * 512],
                                xts_v[fc][:, tt * 128:(tt + 1) * 128],
                                wv_sb[fc][:],
                                start=(fc == 0), stop=(fc == 3),
                            )
                    for k, tt in enumerate((t0, t0 + 1)):
                        va = vaug[tt][:].rearrange("p (h d) -> p h d", d=DK + 1)
                        nc.vector.scalar_tensor_tensor(
                            out=va[:, :, 0:DK],
                            in0=ps[:, k * 512:(k + 1) * 512].rearrange("p (h d) -> p h d", d=DK),
                            scalar=1.0,
                            in1=bv_bcast[:].rearrange("p (h d) -> p h d", d=DK),
                            op0=Alu.mult, op1=Alu.add,
                        )

                def proj_quantum(kind, j, s):
                    # one 512-col slice of a K/Q projection: 4 matmuls + evict
                    for step in proj_micros(kind, j, s):
                        step()

                def proj_micros(kind, j, s):
                    # same work as proj_quantum, as 5 single-op callables so
                    # the P2 interleave never inserts multi-matmul bursts
                    # into the PE's in-order queue
                    xts, w_sb, b_t, dst = (
                        (xts_k, wk_sb, bk_sb, kT_sb[j]) if kind == "k"
                        else (xts_q, wq_sb, bq_sb, qT_sb[j]))
                    state = {}

                    def mm(fc):
                        def f():
                            if fc == 0:
                                state["ps"] = psum_pool.tile(
                                    [128, 512], mybir.dt.float32, tag="w",
                                    name="qkp")
                            nc.tensor.matmul(
                                state["ps"][:],
                                w_sb[fc][:, j * 128:(j + 1) * 128],
                                xts[fc][:, s * 512:(s + 1) * 512],
                                start=(fc == 0), stop=(fc == 3),
                            )
                        return f

                    def ev():
                        nc.vector.tensor_scalar_add(
                            dst[:, s * 512:(s + 1) * 512], state["ps"][:],
                            b_t[:, j:j + 1])

                    return [mm(0), mm(1), mm(2), mm(3), ev]

                def load_mask(qb, c, eng=None):
                    # one DMA: the (1-mask) chunk lands duplicated into both
                    # halves via a step-0 middle dim on the source AP
                    mt = mask_pool.tile([128, 1024], bf16, tag="mask", name="mask")
                    qsl = slice(qb * QBS, (qb + 1) * QBS)
                    src = mbar[c, :, qsl].rearrange("p (r q) -> p r q", r=1).to_broadcast([128, 2, 512])
                    (eng or nc.sync).dma_start(
                        out=mt[:].rearrange("p (r q) -> p r q", r=2), in_=src)
                    return mt

                def p3_micros(qb):
                    qsl = slice(qb * QBS, (qb + 1) * QBS)
                    steps = []
                    for i in range(NFT):
                        state = {}

                        def mm(i, j, state):
                            def f():
                                if j == 0:
                                    state["ps"] = psum_pool.tile(
                                        [128, 512], mybir.dt.float32, tag="w",
                                        name="y")
                                nc.tensor.matmul(
                                    state["ps"][:],
                                    wo_sb[j][:, i * 128:(i + 1) * 128],
                                    o2_sb[(qb, j)][:],
                                    start=(j == 0), stop=(j == NFT - 1),
                                )
                            return f

                        def ev(i, state):
                            def f():
                                y_sb = ysb_pool.tile([128, QBS],
                                                     mybir.dt.float32,
                                                     tag="ysb", bufs=4,
                                                     name="ysb")
                                nc.vector.tensor_scalar_add(
                                    y_sb[:], state["ps"][:], bo_sb[:, i:i + 1])
                                nc.sync.dma_start(
                                    out=yT[i * 128:(i + 1) * 128, qsl],
                                    in_=y_sb[:])
                            return f

                        for j in range(NFT):
                            steps.append(mm(i, j, state))
                        steps.append(ev(i, state))
                    return steps

                def p3(qb):
                    for step in p3_micros(qb):
                        step()

                def epilogue_micros(qb, j, idx, o_ps):
                    # divide O rows by the denominator (row DK), write o2.
                    # Broadcast the denom row to 64 partitions with a K=1
                    # matmul (ones ⊗ row) — no DRAM bounce — then a fast
                    # custom-DVE reciprocal on the broadcast.  Returned as
                    # micro-steps so the next block's scores matmuls sit
                    # ahead of the bc matmul in the PE's in-order queue.
                    state = {}

                    def s1():
                        state["dn"] = rb_pool.tile([65, QBS], bf16, tag="dn",
                                                   bufs=2, name="dn")
                        nc.vector.tensor_copy(state["dn"][64:65, :],
                                              o_ps[DK:DK + 1, :])
                        state["bc"] = psum_pool.tile(
                            [128, 512], mybir.dt.float32, tag="w", name="bc")
                        nc.tensor.matmul(state["bc"][0:64, :],
                                         ones_sb[64:65, 0:DK],
                                         state["dn"][64:65, :],
                                         start=True, stop=True)

                    def s2():
                        state["rb"] = rb_pool.tile([64, QBS],
                                                   mybir.dt.float32, tag="rb",
                                                   bufs=2, name="rb")
                        nc.vector.reciprocal_approx_fast(state["rb"][:],
                                                         state["bc"][0:64, :])

                    def s3():
                        if idx == 0:
                            nc.vector.tensor_mul(o2_sb[(qb, j)][0:64, :],
                                                 o_ps[0:DK, :], state["rb"][:])
                        else:
                            osm = rb_pool.tile([64, QBS], bf16, tag="osm",
                                               bufs=2, name="osm")
                            nc.vector.tensor_mul(osm[:], o_ps[0:DK, :],
                                                 state["rb"][:])
                            nc.sync.dma_start(out=o2_sb[(qb, j)][64:128, :],
                                              in_=osm[:])

                    return [s1, s2, s3]

                def epilogue(qb, j, idx, o_ps):
                    for step in epilogue_micros(qb, j, idx, o_ps):
                        step()

                # ============ P1: K(0), V full, Q(0) ============
                mask_t = {}
                for c in range(8):
                    mask_t[(0, c)] = load_mask(0, c)
                for b_dram, b_t in ((bq, bq_sb), (bk, bk_sb), (bo, bo_sb)):
                    nc.sync.dma_start(out=b_t[:], in_=b_dram.ap().rearrange("(j p) -> p j", p=128))
                for j in range(NFT):
                    nc.sync.dma_start(out=wo_sb[j][:], in_=woT[j * 128:(j + 1) * 128, :])
                for c in range(8, NCH):
                    mask_t[(0, c)] = load_mask(0, c)

                # warm the PE's HAM clock gate with tiny matmuls while the
                # input DMAs stream, so the projections run at 2.4 GHz
                warm_ps = psum_pool.tile([128, 512], mybir.dt.float32,
                                         tag="w", name="warm")
                for _ in range(60):
                    nc.tensor.matmul(warm_ps[0:64, 0:64],
                                     ones_sb[0:64, 0:DK],
                                     ones_sb[0:64, 0:DK],
                                     start=True, stop=True)

                for t0 in range(0, NCH, 2):
                    v_proj_pair(t0)
                for s in range(4):
                    proj_quantum("k", 0, s)
                for s in range(4):
                    proj_quantum("q", 0, s)

                # interleave schedule: single-instruction micro-steps issued
                # 2-per-chunk inside each (qb, pair) block.  K(j) fully before
                # block (0, j); Q(j, qb') before block (qb', j); p3(qb) in
                # block (qb+1, 0).
                interleave = {
                    (0, 0): [("k", 1, 0), ("k", 1, 1), ("k", 1, 2), ("k", 1, 3), ("q", 1, 0)],
                    (0, 1): [("k", 2, 0), ("k", 2, 1), ("k", 2, 2), ("k", 2, 3), ("q", 2, 0)],
                    (0, 2): [("k", 3, 0), ("k", 3, 1), ("k", 3, 2), ("k", 3, 3), ("q", 3, 0)],
                    (0, 3): [("q", 1, 1), ("q", 2, 1), ("q", 3, 1)],
                    (1, 0): [("p3", 0, 0), ("q", 1, 2)],
                    (1, 1): [("q", 2, 2)],
                    (1, 2): [("q", 3, 2)],
                    (1, 3): [("q", 1, 3)],
                    (2, 0): [("p3", 1, 0), ("q", 2, 3)],
                    (2, 1): [("q", 3, 3)],
                    (3, 0): [("p3", 2, 0)],
                }

                # ============ P2 ============
                for qb in range(QB):
                    for j in range(NFT):  # head pair j -> heads 2j, 2j+1
                        steps = []
                        for kind, a1, a2 in interleave.get((qb, j), []):
                            if kind == "p3":
                                steps.extend(p3_micros(a1))
                            else:
                                steps.extend(proj_micros(kind, a1, a2))
                        o_ps = {}
                        for idx in range(2):
                            o_ps[idx] = psum_pool.tile(
                                [DK + 1, QBS], mybir.dt.float32,
                                tag="o", name="o")
                        for c in range(NCH):
                            for _ in range(2):
                                if steps:
                                    steps.pop(0)()
                            # mask prefetch for the next qb during pair 2 —
                            # keeps the Sync DMA queue clear of bulk work
                            # when pair 3's epilogue o2 DMA is issued
                            if j == 2 and qb < QB - 1:
                                mask_t[(qb + 1, c)] = load_mask(qb + 1, c)

                            sp = psum_pool.tile([128, 1024], mybir.dt.float32,
                                                tag="s", name="s")
                            for idx in range(2):
                                rsl = slice(idx * 64, idx * 64 + 64)
                                nc.tensor.matmul(
                                    sp[:, idx * 512:(idx + 1) * 512],
                                    kT_sb[j][rsl, c * 128:(c + 1) * 128],
                                    qT_sb[j][rsl, qb * QBS:(qb + 1) * QBS],
                                    start=True, stop=True,
                                )
                            p_raw = praw_pool.tile([128, 1024], bf16,
                                                   tag="praw", name="praw")
                            nc.scalar.activation(p_raw[:], sp[:], Exp,
                                                 bias=0.0, scale=0.125)
                            p_m = pm_pool.tile([128, 1024], bf16, tag="pm", name="pm")
                            nc.vector.tensor_mul(p_m[:], p_raw[:], mask_t[(qb, c)][:])
                            for idx in range(2):
                                h = 2 * j + idx
                                nc.tensor.matmul(
                                    o_ps[idx][:],
                                    vaug[c][:, h * (DK + 1):(h + 1) * (DK + 1)],
                                    p_m[:, idx * 512:(idx + 1) * 512],
                                    start=(c == 0), stop=(c == NCH - 1),
                                )
                        for step in steps:  # any leftovers
                            step()
                        # idx=1 first: its o2 write needs a DMA hop
                        for idx in (1, 0):
                            epilogue(qb, j, idx, o_ps[idx])

                p3(QB - 1)

    nc.compile()
    return nc


def _get_nc():
    if "nc" not in _cache:
        _cache["nc"] = _build_nc()
    return _cache["nc"]


def _make_in_maps(inputs):
    query = np.asarray(inputs["query"], np.float32)
    key = np.asarray(inputs["key"], np.float32)
    value = np.asarray(inputs["value"], np.float32)
    mask = np.asarray(inputs["mask"], bool)
    shared = {
        "wqT": np.ascontiguousarray(np.asarray(inputs["Wq"], np.float32).T).astype(BF16),
        "wkT": np.ascontiguousarray(np.asarray(inputs["Wk"], np.float32).T).astype(BF16),
        "wvT": np.ascontiguousarray(np.asarray(inputs["Wv"], np.float32).T).astype(BF16),
        "woT": np.ascontiguousarray(np.asarray(inputs["Wo"], np.float32).T).astype(BF16),
        "bq": np.asarray(inputs["bq"], np.float32),
        "bk": np.asarray(inputs["bk"], np.float32),
        "bv": np.asarray(inputs["bv"], np.float32),
        "bo": np.asarray(inputs["bo"], np.float32),
    }
    in_maps = []
    for b in range(N_CORES):
        m = dict(shared)
        m["xqT"] = np.ascontiguousarray(query[b].T).astype(BF16)
        m["xkT"] = np.ascontiguousarray(key[b].T).astype(BF16)
        m["xvT"] = np.ascontiguousarray(value[b].T).astype(BF16)
        mb = (~mask[b]).T.astype(BF16)          # (1 - mask)^T, [t2, q]
        m["mbar"] = np.ascontiguousarray(mb.reshape(NCH, 128, T))
        in_maps.append(m)
    return in_maps


def run(inputs, trace=False, **kwargs):
    from concourse.bass_utils import run_bass_kernel_spmd
    nc = _get_nc()
    res = run_bass_kernel_spmd(nc, _make_in_maps(inputs),
                               core_ids=list(range(N_CORES)),
                               trace=trace, **kwargs)
    y = np.stack([np.asarray(res.results[b]["yT"], np.float32).T
                  for b in range(N_CORES)])
    return y, res


def kernel(**inputs) -> np.ndarray:
    y, _ = run(inputs, trace=False)
    return y


# revision 43
# speedup vs baseline: 1.0062x; 1.0062x over previous
"""Multi-head attention (B=8, T=2048, D=512, H=8) on 8 TRN2 NeuronCores.

Sharding: data-parallel over batch — one batch element per core, no
collectives. Host-side prep (part of shard/unshard): transpose x inputs to
[D, T], cast matmul operands to bf16, pass (1 - mask)^T chunk-major, and
transpose the per-core output y^T back to [T, D].

Per-core algorithm ("transposed flash"), v2 — head-pair-packed scores:
  P1: Q^T = Wq x^T and K^T = Wk x^T stored pair-major ([128, T] tiles, head
      2j in rows 0-63, head 2j+1 in rows 64-127 — no zero padding);
      V = x Wv^T augmented with a ones column per head (softmax denom).
  P2: per (q-block of 512, head-pair j, t2-chunk c):
        two 64-contraction score matmuls run CONCURRENTLY on the PE via
        tile_position (0,0)/(64,0), writing the two halves of one
        [128, 1024] PSUM tile; one exp ACTIVATE covers the pair; one DVE
        multiply applies the (1-mask) chunk (duplicated into both halves);
        per-head attnV accumulation into [65, 512] PSUM (row 64 = denom).
      epilogue per head: recip(denom) via 8-partition split + DRAM bounce
      broadcast, multiply into o2 SBUF.
  P3: y^T = Wo^T.T @ O^T (+bo) per q-block, DMA out.

PSUM budget (8 banks): tag s = 2x[128,1024] (4), tag o = 2x[65,512] (2),
tag w = 2x[128,512] (2, dedicated to projections + P3 so they never starve
the softmax pipeline). K/Q projections for j>0 are interleaved into P2 in
4-matmul quanta; P3(qb) is interleaved into block (qb+1, 0).
"""

import numpy as np
import ml_dtypes

B, T, FDIM, H = 8, 2048, 512, 8
DK = FDIM // H          # 64
NFT = FDIM // 128       # 4 fo-tiles
NCH = T // 128          # 16 t2-chunks
QB = 4                  # q blocks
QBS = T // QB           # 512
N_CORES = 8

BF16 = ml_dtypes.bfloat16

_cache = {}


def _build_nc():
    import concourse.bass as bass
    import concourse.mybir as mybir
    from concourse import bacc, tile

    f32 = mybir.dt.float32
    bf16 = mybir.dt.bfloat16
    Exp = mybir.ActivationFunctionType.Exp
    Alu = mybir.AluOpType

    nc = bacc.Bacc("TRN2", target_bir_lowering=False, debug=False,
                   num_devices=N_CORES)

    # DRAM I/O (per-core shard shapes)
    xqT = nc.dram_tensor("xqT", [FDIM, T], bf16, kind="ExternalInput")
    xkT = nc.dram_tensor("xkT", [FDIM, T], bf16, kind="ExternalInput")
    xvT = nc.dram_tensor("xvT", [FDIM, T], bf16, kind="ExternalInput")
    wqT = nc.dram_tensor("wqT", [FDIM, FDIM], bf16, kind="ExternalInput")
    wkT = nc.dram_tensor("wkT", [FDIM, FDIM], bf16, kind="ExternalInput")
    wvT = nc.dram_tensor("wvT", [FDIM, FDIM], bf16, kind="ExternalInput")
    woT = nc.dram_tensor("woT", [FDIM, FDIM], bf16, kind="ExternalInput")
    bq = nc.dram_tensor("bq", [FDIM], f32, kind="ExternalInput")
    bk = nc.dram_tensor("bk", [FDIM], f32, kind="ExternalInput")
    bv = nc.dram_tensor("bv", [FDIM], f32, kind="ExternalInput")
    bo = nc.dram_tensor("bo", [FDIM], f32, kind="ExternalInput")
    mbar = nc.dram_tensor("mbar", [NCH, 128, T], bf16, kind="ExternalInput")
    yT = nc.dram_tensor("yT", [FDIM, T], f32, kind="ExternalOutput")

    with tile.TileContext(nc) as tc:
        with (
            tc.tile_pool(name="consts", bufs=1) as consts,
            tc.tile_pool(name="qt", bufs=1) as qt_pool,
            tc.tile_pool(name="kt", bufs=1) as kt_pool,
            tc.tile_pool(name="vaug", bufs=1) as vaug_pool,
            tc.tile_pool(name="osb", bufs=1) as osb_pool,
            tc.tile_pool(name="ysb", bufs=1) as ysb_pool,
        ):
            # ---- consts: weights + biases (wv/xv first: head of critical path)
            wv_sb = [consts.tile([128, FDIM], bf16, tag=f"wv{fc}", name=f"wv{fc}") for fc in range(4)]
            wk_sb = [consts.tile([128, FDIM], bf16, tag=f"wk{fc}", name=f"wk{fc}") for fc in range(4)]
            wq_sb = [consts.tile([128, FDIM], bf16, tag=f"wq{fc}", name=f"wq{fc}") for fc in range(4)]
            wo_sb = [consts.tile([128, FDIM], bf16, tag=f"wo{j}", name=f"wo{j}") for j in range(NFT)]

            ones_sb = consts.tile([128, DK], bf16, tag="ones", name="ones")
            bq_sb = consts.tile([128, NFT], f32, tag="bq", name="bq")
            bk_sb = consts.tile([128, NFT], f32, tag="bk", name="bk")
            bo_sb = consts.tile([128, NFT], f32, tag="bo", name="bo")
            bv_bcast = consts.tile([128, FDIM], f32, tag="bv_bcast", name="bv_bcast")

            # ---- persistent activation tiles ----
            qT_sb = [qt_pool.tile([128, T], bf16, tag=f"qT{j}", name=f"qT{j}") for j in range(NFT)]
            kT_sb = [kt_pool.tile([128, T], bf16, tag=f"kT{j}", name=f"kT{j}") for j in range(NFT)]
            vaug = [vaug_pool.tile([128, H * (DK + 1)], bf16, tag=f"va{tt}", name=f"va{tt}")
                    for tt in range(NCH)]
            o2_sb = {}
            for qb in range(QB):
                for j in range(NFT):
                    o2_sb[(qb, j)] = osb_pool.tile([128, QBS], bf16, tag=f"o2_{qb}_{j}",
                                                   name=f"o2_{qb}_{j}")

            with (
                tc.tile_pool(name="xt", bufs=12) as xt_pool,
                tc.tile_pool(name="mask", bufs=20) as mask_pool,
                tc.tile_pool(name="praw", bufs=4) as praw_pool,
                tc.tile_pool(name="pm", bufs=5) as pm_pool,
                tc.tile_pool(name="rb", bufs=1) as rb_pool,
                tc.tile_pool(name="psum", bufs=2, space="PSUM") as psum_pool,
            ):
                # ============ DMA: inputs in critical-path order ============
                def load_x(x_dram, nm):
                    tiles = []
                    for fc in range(4):
                        xt = xt_pool.tile([128, T], bf16, tag="x", bufs=12, name=nm)
                        nc.sync.dma_start(out=xt[:], in_=x_dram[fc * 128:(fc + 1) * 128, :])
                        tiles.append(xt)
                    return tiles

                # Sync queue: K then V inputs.  Scalar queue (the second
                # HWDGE engine, idle during the head) issues Q inputs,
                # biases, Wo and the first q-block's masks in parallel —
                # DMA issue costs ~0.65us apiece, so one queue would
                # serialize ~35us of issue time.
                for fc in range(4):
                    nc.sync.dma_start(out=wv_sb[fc][:], in_=wvT[fc * 128:(fc + 1) * 128, :])
                xts_v = load_x(xvT, "xv")
                nc.sync.dma_start(
                    out=bv_bcast[:],
                    in_=bv.ap().rearrange("(a f) -> a f", a=1).to_broadcast([128, FDIM]))
                for fc in range(4):
                    nc.sync.dma_start(out=wk_sb[fc][:], in_=wkT[fc * 128:(fc + 1) * 128, :])
                xts_k = load_x(xkT, "xk")
                for fc in range(4):
                    nc.sync.dma_start(out=wq_sb[fc][:], in_=wqT[fc * 128:(fc + 1) * 128, :])
                xts_q = load_x(xqT, "xq")

                # ones column per head in V_aug
                nc.vector.memset(ones_sb[:], 1.0)
                for tt in range(NCH):
                    va = vaug[tt][:].rearrange("p (h d) -> p h d", d=DK + 1)
                    nc.vector.memset(va[:, :, DK:DK + 1], 1.0)

                # ============ compute helpers ============
                def v_proj_pair(t0):
                    # two V-proj tiles through one [128,1024] "s" slot
                    ps = psum_pool.tile([128, 1024], mybir.dt.float32,
                                        tag="s", name="vp")
                    for k, tt in enumerate((t0, t0 + 1)):
                        for fc in range(4):
                            nc.tensor.matmul(
                                ps[:, k * 512:(k + 1) * 512],
                                xts_v[fc][:, tt * 128:(tt + 1) * 128],
                                wv_sb[fc][:],
                                start=(fc == 0), stop=(fc == 3),
                            )
                    for k, tt in enumerate((t0, t0 + 1)):
                        va = vaug[tt][:].rearrange("p (h d) -> p h d", d=DK + 1)
                        nc.vector.scalar_tensor_tensor(
                            out=va[:, :, 0:DK],
                            in0=ps[:, k * 512:(k + 1) * 512].rearrange("p (h d) -> p h d", d=DK),
                            scalar=1.0,
                            in1=bv_bcast[:].rearrange("p (h d) -> p h d", d=DK),
                            op0=Alu.mult, op1=Alu.add,
                        )

                def proj_quantum(kind, j, s):
                    # one 512-col slice of a K/Q projection: 4 matmuls + evict
                    for step in proj_micros(kind, j, s):
                        step()

                def proj_micros(kind, j, s):
                    # same work as proj_quantum, as 5 single-op callables so
                    # the P2 interleave never inserts multi-matmul bursts
                    # into the PE's in-order queue
                    xts, w_sb, b_t, dst = (
                        (xts_k, wk_sb, bk_sb, kT_sb[j]) if kind == "k"
                        else (xts_q, wq_sb, bq_sb, qT_sb[j]))
                    state = {}

                    def mm(fc):
                        def f():
                            if fc == 0:
                                state["ps"] = psum_pool.tile(
                                    [128, 512], mybir.dt.float32, tag="w",
                                    name="qkp")
                            nc.tensor.matmul(
                                state["ps"][:],
                                w_sb[fc][:, j * 128:(j + 1) * 128],
                                xts[fc][:, s * 512:(s + 1) * 512],
                                start=(fc == 0), stop=(fc == 3),
                            )
                        return f

                    def ev():
                        nc.vector.tensor_scalar_add(
                            dst[:, s * 512:(s + 1) * 512], state["ps"][:],
                            b_t[:, j:j + 1])

                    return [mm(0), mm(1), mm(2), mm(3), ev]

                def load_mask(qb, c, eng=None):
                    # one DMA: the (1-mask) chunk lands duplicated into both
                    # halves via a step-0 middle dim on the source AP
                    mt = mask_pool.tile([128, 1024], bf16, tag="mask", name="mask")
                    qsl = slice(qb * QBS, (qb + 1) * QBS)
                    src = mbar[c, :, qsl].rearrange("p (r q) -> p r q", r=1).to_broadcast([128, 2, 512])
                    (eng or nc.sync).dma_start(
                        out=mt[:].rearrange("p (r q) -> p r q", r=2), in_=src)
                    return mt

                def p3_micros(qb):
                    qsl = slice(qb * QBS, (qb + 1) * QBS)
                    steps = []
                    for i in range(NFT):
                        state = {}

                        def mm(i, j, state):
                            def f():
                                if j == 0:
                                    state["ps"] = psum_pool.tile(
                                        [128, 512], mybir.dt.float32, tag="w",
                                        name="y")
                                nc.tensor.matmul(
                                    state["ps"][:],
                                    wo_sb[j][:, i * 128:(i + 1) * 128],
                                    o2_sb[(qb, j)][:],
                                    start=(j == 0), stop=(j == NFT - 1),
                                )
                            return f

                        def ev(i, state):
                            def f():
                                y_sb = ysb_pool.tile([128, QBS],
                                                     mybir.dt.float32,
                                                     tag="ysb", bufs=4,
                                                     name="ysb")
                                nc.vector.tensor_scalar_add(
                                    y_sb[:], state["ps"][:], bo_sb[:, i:i + 1])
                                nc.sync.dma_start(
                                    out=yT[i * 128:(i + 1) * 128, qsl],
                                    in_=y_sb[:])
                            return f

                        for j in range(NFT):
                            steps.append(mm(i, j, state))
                        steps.append(ev(i, state))
                    return steps

                def p3(qb):
                    for step in p3_micros(qb):
                        step()

                def epilogue_micros(qb, j, idx, o_ps):
                    # divide O rows by the denominator (row DK), write o2.
                    # Broadcast the denom row to 64 partitions with a K=1
                    # matmul (ones ⊗ row) — no DRAM bounce — then a fast
                    # custom-DVE reciprocal on the broadcast.  Returned as
                    # micro-steps so the next block's scores matmuls sit
                    # ahead of the bc matmul in the PE's in-order queue.
                    state = {}

                    def s1():
                        state["dn"] = rb_pool.tile([65, QBS], bf16, tag="dn",
                                                   bufs=2, name="dn")
                        nc.vector.tensor_copy(state["dn"][64:65, :],
                                              o_ps[DK:DK + 1, :])
                        state["bc"] = psum_pool.tile(
                            [128, 512], mybir.dt.float32, tag="w", name="bc")
                        nc.tensor.matmul(state["bc"][0:64, :],
                                         ones_sb[64:65, 0:DK],
                                         state["dn"][64:65, :],
                                         start=True, stop=True)

                    def s2():
                        state["rb"] = rb_pool.tile([64, QBS],
                                                   mybir.dt.float32, tag="rb",
                                                   bufs=2, name="rb")
                        nc.vector.reciprocal_approx_fast(state["rb"][:],
                                                         state["bc"][0:64, :])

                    def s3():
                        if idx == 0:
                            nc.vector.tensor_mul(o2_sb[(qb, j)][0:64, :],
                                                 o_ps[0:DK, :], state["rb"][:])
                        else:
                            osm = rb_pool.tile([64, QBS], bf16, tag="osm",
                                               bufs=2, name="osm")
                            nc.vector.tensor_mul(osm[:], o_ps[0:DK, :],
                                                 state["rb"][:])
                            nc.sync.dma_start(out=o2_sb[(qb, j)][64:128, :],
                                              in_=osm[:])

                    return [s1, s2, s3]

                def epilogue(qb, j, idx, o_ps):
                    for step in epilogue_micros(qb, j, idx, o_ps):
                        step()

                # ============ P1: K(0), V full, Q(0) ============
                mask_t = {}
                for c in range(8):
                    mask_t[(0, c)] = load_mask(0, c)
                for b_dram, b_t in ((bq, bq_sb), (bk, bk_sb), (bo, bo_sb)):
                    nc.sync.dma_start(out=b_t[:], in_=b_dram.ap().rearrange("(j p) -> p j", p=128))
                for j in range(NFT):
                    nc.sync.dma_start(out=wo_sb[j][:], in_=woT[j * 128:(j + 1) * 128, :])
                for c in range(8, NCH):
                    mask_t[(0, c)] = load_mask(0, c)

                # warm the PE's HAM clock gate with tiny matmuls while the
                # input DMAs stream, so the projections run at 2.4 GHz
                warm_ps = psum_pool.tile([128, 512], mybir.dt.float32,
                                         tag="w", name="warm")
                for _ in range(60):
                    nc.tensor.matmul(warm_ps[0:64, 0:64],
                                     ones_sb[0:64, 0:DK],
                                     ones_sb[0:64, 0:DK],
                                     start=True, stop=True)

                # only the slices the first block needs: kT[0] cols stream in
                # chunk order (slices s>=1 interleave into block (0,0)) and
                # qb=0 reads qT[:, 0:512] only — all other q-slices are
                # interleaved just-in-time before the q-block that reads them
                for t0 in range(0, NCH, 2):
                    v_proj_pair(t0)
                proj_quantum("k", 0, 0)
                proj_quantum("q", 0, 0)

                # interleave schedule: single-instruction micro-steps issued
                # 2-per-chunk inside each (qb, pair) block.  K(j) fully before
                # block (0, j); Q(j, qb') before block (qb', j); p3(qb) in
                # block (qb+1, 0).
                interleave = {
                    (0, 0): [("k", 0, 1), ("k", 0, 2), ("k", 0, 3),
                             ("k", 1, 0), ("k", 1, 1), ("k", 1, 2), ("k", 1, 3), ("q", 1, 0)],
                    (0, 1): [("k", 2, 0), ("k", 2, 1), ("k", 2, 2), ("k", 2, 3), ("q", 2, 0)],
                    (0, 2): [("k", 3, 0), ("k", 3, 1), ("k", 3, 2), ("k", 3, 3), ("q", 3, 0)],
                    (0, 3): [("q", 0, 1), ("q", 1, 1), ("q", 2, 1), ("q", 3, 1)],
                    (1, 0): [("p3", 0, 0)],
                    (1, 3): [("q", 0, 2), ("q", 1, 2), ("q", 2, 2), ("q", 3, 2)],
                    (2, 0): [("p3", 1, 0)],
                    (2, 3): [("q", 0, 3), ("q", 1, 3), ("q", 2, 3), ("q", 3, 3)],
                    (3, 0): [("p3", 2, 0)],
                }

                # ============ P2 ============
                for qb in range(QB):
                    for j in range(NFT):  # head pair j -> heads 2j, 2j+1
                        steps = []
                        for kind, a1, a2 in interleave.get((qb, j), []):
                            if kind == "p3":
                                steps.extend(p3_micros(a1))
                            else:
                                steps.extend(proj_micros(kind, a1, a2))
                        o_ps = {}
                        for idx in range(2):
                            o_ps[idx] = psum_pool.tile(
                                [DK + 1, QBS], mybir.dt.float32,
                                tag="o", name="o")
                        # block (0,0) carries 40 micro-steps (K0 tail + K1 +
                        # Q1) — pop faster there; its kT[0] slice s arrives
                        # just ahead of the chunks (c >= 4s) that read it
                        pop_n = 3 if (qb, j) == (0, 0) else 2
                        for c in range(NCH):
                            for _ in range(pop_n):
                                if steps:
                                    steps.pop(0)()
                            # mask prefetch for the next qb during pair 2 —
                            # keeps the Sync DMA queue clear of bulk work
                            # when pair 3's epilogue o2 DMA is issued
                            if j == 2 and qb < QB - 1:
                                mask_t[(qb + 1, c)] = load_mask(qb + 1, c)

                            sp = psum_pool.tile([128, 1024], mybir.dt.float32,
                                                tag="s", name="s")
                            for idx in range(2):
                                rsl = slice(idx * 64, idx * 64 + 64)
                                nc.tensor.matmul(
                                    sp[:, idx * 512:(idx + 1) * 512],
                                    kT_sb[j][rsl, c * 128:(c + 1) * 128],
                                    qT_sb[j][rsl, qb * QBS:(qb + 1) * QBS],
                                    start=True, stop=True,
                                )
                            p_raw = praw_pool.tile([128, 1024], bf16,
                                                   tag="praw", name="praw")
                            nc.scalar.activation(p_raw[:], sp[:], Exp,
                                                 bias=0.0, scale=0.125)
                            p_m = pm_pool.tile([128, 1024], bf16, tag="pm", name="pm")
                            nc.vector.tensor_mul(p_m[:], p_raw[:], mask_t[(qb, c)][:])
                            for idx in range(2):
                                h = 2 * j + idx
                                nc.tensor.matmul(
                                    o_ps[idx][:],
                                    vaug[c][:, h * (DK + 1):(h + 1) * (DK + 1)],
                                    p_m[:, idx * 512:(idx + 1) * 512],
                                    start=(c == 0), stop=(c == NCH - 1),
                                )
                        for step in steps:  # any leftovers
                            step()
                        # idx=1 first: its o2 write needs a DMA hop
                        for idx in (1, 0):
                            epilogue(qb, j, idx, o_ps[idx])

                p3(QB - 1)

    nc.compile()
    return nc


def _get_nc():
    if "nc" not in _cache:
        _cache["nc"] = _build_nc()
    return _cache["nc"]


def _make_in_maps(inputs):
    query = np.asarray(inputs["query"], np.float32)
    key = np.asarray(inputs["key"], np.float32)
    value = np.asarray(inputs["value"], np.float32)
    mask = np.asarray(inputs["mask"], bool)
    shared = {
        "wqT": np.ascontiguousarray(np.asarray(inputs["Wq"], np.float32).T).astype(BF16),
        "wkT": np.ascontiguousarray(np.asarray(inputs["Wk"], np.float32).T).astype(BF16),
        "wvT": np.ascontiguousarray(np.asarray(inputs["Wv"], np.float32).T).astype(BF16),
        "woT": np.ascontiguousarray(np.asarray(inputs["Wo"], np.float32).T).astype(BF16),
        "bq": np.asarray(inputs["bq"], np.float32),
        "bk": np.asarray(inputs["bk"], np.float32),
        "bv": np.asarray(inputs["bv"], np.float32),
        "bo": np.asarray(inputs["bo"], np.float32),
    }
    in_maps = []
    for b in range(N_CORES):
        m = dict(shared)
        m["xqT"] = np.ascontiguousarray(query[b].T).astype(BF16)
        m["xkT"] = np.ascontiguousarray(key[b].T).astype(BF16)
        m["xvT"] = np.ascontiguousarray(value[b].T).astype(BF16)
        mb = (~mask[b]).T.astype(BF16)          # (1 - mask)^T, [t2, q]
        m["mbar"] = np.ascontiguousarray(mb.reshape(NCH, 128, T))
        in_maps.append(m)
    return in_maps


def run(inputs, trace=False, **kwargs):
    from concourse.bass_utils import run_bass_kernel_spmd
    nc = _get_nc()
    res = run_bass_kernel_spmd(nc, _make_in_maps(inputs),
                               core_ids=list(range(N_CORES)),
                               trace=trace, **kwargs)
    y = np.stack([np.asarray(res.results[b]["yT"], np.float32).T
                  for b in range(N_CORES)])
    return y, res


def kernel(**inputs) -> np.ndarray:
    y, _ = run(inputs, trace=False)
    return y
